# revision 1
# baseline (speedup 1.0000x reference)
"""Trainium2 Bass kernel for nn_Bernprop2 (BernNet-style GNN propagation).

Strategy (see sharding_hint): destination-node sharding across 8 cores.
Each SpMM stage: dma_gather source rows (int16 indices into 2-rank chunks)
-> one-hot S matrices built on DVE (is_equal vs iota, weights folded in)
-> TensorE matmul segment-sum accumulating per 128-row window in PSUM
-> per-window copy into an SBUF accumulator. Inter-stage tables are
exchanged with ncfw AllGather into internal Shared DRAM.

Tables live in a permuted "device layout": node n -> slot
k*RP + p*W + w  (k=n//R, r=n%R, w=r//P, p=r%P) so every table write is one
contiguous DMA and gather indices within a 2-rank chunk fit in int16.
"""

import sys

if "/opt/trn_rl_repo" not in sys.path:
    sys.path.insert(0, "/opt/trn_rl_repo")

import numpy as np

P = 128  # partitions / window rows / tile edges
V_MUL = False  # fold edge weights into V (True) or into S (False)


class Cfg:
    def __init__(self, N=100000, E=1250000, D=64, C=8, block_w=8,
                 n_queues=4):
        self.N, self.E, self.D, self.C = N, E, D, C
        self.NQ = n_queues
        assert N % C == 0
        self.R = N // C                     # rows per core
        self.W = -(-self.R // P)            # windows per core
        self.RP = self.W * P                # padded rows per core
        self.NP = self.C * self.RP          # padded table rows
        self.CHUNK = 2 * self.RP            # rows per gather chunk (2 ranks)
        assert self.CHUNK <= 32767
        self.NCH = self.C // 2              # number of chunks
        self.BLOCK_W = block_w              # windows per block
        self.NBLK = -(-self.W // self.BLOCK_W)


def _slot(cfg, n):
    """Global device-table slot for node id array n."""
    k = n // cfg.R
    r = n - k * cfg.R
    return k * cfg.RP + (r % P) * cfg.W + (r // P)


def _chunk_idx(cfg, n):
    """(chunk id, int16 index within chunk) for source node array n."""
    k = n // cfg.R
    r = n - k * cfg.R
    return (k >> 1), (k & 1) * cfg.RP + (r % P) * cfg.W + (r // P)


def _to_dev_table(cfg, x):
    """[N, D] -> [NP, D] permuted device table."""
    out = np.zeros((cfg.NP, x.shape[1]), dtype=x.dtype)
    out[_slot(cfg, np.arange(cfg.N))] = x
    return out


def _from_dev_rows(cfg, a):
    """[P, W*D] per-core device rows -> [R, D]."""
    full = a.reshape(P, cfg.W, cfg.D).transpose(1, 0, 2).reshape(cfg.RP, cfg.D)
    return full[: cfg.R]


class Graph:
    """Shared schedule + per-core blobs for one edge list."""

    def __init__(self, cfg, row, col, wv):
        C, R, W, NCH, BW = cfg.C, cfg.R, cfg.W, cfg.NCH, cfg.BLOCK_W
        per_core = []
        counts = np.zeros((C, NCH, W), np.int64)
        for k in range(C):
            m = (row >= k * R) & (row < (k + 1) * R)
            r = row[m] - k * R
            cc, gi = _chunk_idx(cfg, col[m])
            win, ld = r // P, r % P
            order = np.lexsort((gi, ld, win, cc, win // BW))
            per_core.append((cc[order], win[order], ld[order], gi[order],
                             wv[m][order]))
            np.add.at(counts[k], (cc[order], win[order]), 1)
        maxc = counts.max(axis=0)                      # [NCH, W]
        ntile = -(-maxc // P)                          # tiles per (c, w) cell
        ntile[0] = np.maximum(ntile[0], 1)             # c0 owns start=True

        # Schedule: blocks -> cells (c, list of (w, ntiles)) in stream order.
        self.blocks = []
        tot = 0
        for b in range(cfg.NBLK):
            ws = range(b * BW, min((b + 1) * BW, W))
            cells = []
            for c in range(NCH):
                wt = [(w, int(ntile[c, w])) for w in ws if ntile[c, w] > 0]
                n = sum(t for _, t in wt) * P
                cells.append((c, tot, n, wt))
                tot += n
            self.blocks.append((list(ws), cells))
        self.total = tot
        self.max_cell = max((n for _, (_, cells) in enumerate(self.blocks)
                             for (_, _, n, _) in cells), default=0)
        self.max_blk = max((sum(n for (_, _, n, _) in cells)
                            for _, cells in self.blocks), default=0)

        # Per-core blobs in schedule layout.
        self.gidx, self.ldw, self.wvv = [], [], []
        for k in range(C):
            cc, win, ld, gi, wv_ = per_core[k]
            g16 = np.zeros(tot, np.int16)
            ldf = np.zeros(tot, np.float32)
            wvf = np.zeros(tot, np.float32)
            # cell start offsets for this core's edges
            pos = 0
            starts = {}
            for ws_, cells in self.blocks:
                for (c, off, n, wt) in cells:
                    o = off
                    for (w, t) in wt:
                        starts[(c, w)] = o
                        o += t * P
            # place edges: within (c, w) contiguous, stream-sorted already
            keys = cc * W + win
            uk, first, cnt = np.unique(keys, return_index=True,
                                       return_counts=True)
            for u, f, n_ in zip(uk, first, cnt):
                c, w = int(u) // W, int(u) % W
                o = starts[(c, w)]
                g16[o:o + n_] = gi[f:f + n_]
                ldf[o:o + n_] = ld[f:f + n_]
                wvf[o:o + n_] = wv_[f:f + n_]
            # wrapped/interleaved device layouts
            self.gidx.append(np.tile(g16.reshape(-1, 16).T, (8, 1)).copy())
            self.ldw.append(ldf.reshape(-1, P).T.copy())
            self.wvv.append(wvf.reshape(-1, P).T.copy())


def _emulate_stage(cfg, g, table_dev, core):
    """Numpy emulation of one SpMM stage -> [P, W*D] device rows."""
    acc = np.zeros((P, cfg.W, cfg.D), np.float32)
    gi, ld, wv = g.gidx[core][:16], g.ldw[core], g.wvv[core]
    for ws, cells in g.blocks:
        for (c, off, n, wt) in cells:
            o = off
            for (w, t) in wt:
                for ti in range(t):
                    e0 = o + ti * P
                    idx = gi[:, e0 // 16:(e0 + P) // 16].T.reshape(-1)
                    V = table_dev[c * cfg.CHUNK + idx.astype(np.int64)]
                    S = np.zeros((P, P), np.float32)
                    S[np.arange(P), ld[:, e0 // P].astype(np.int64)] = \
                        wv[:, e0 // P]
                    acc[:, w] += S.T @ V
                o += t * P
    return acc.reshape(P, cfg.W * cfg.D)


# ---------------------------------------------------------------- builder --

def build_program(cfg, graphs, repeat=1, variant="full"):
    """graphs = dict(L=Graph, NB=Graph, NS=Graph). Returns compiled nc."""
    import concourse.bacc as bacc
    import concourse.mybir as mybir
    import concourse.tile as tile

    D, W, NP, CHUNK, NCH = cfg.D, cfg.W, cfg.NP, cfg.CHUNK, cfg.NCH
    f32 = mybir.dt.float32
    nc = bacc.Bacc("TRN2", target_bir_lowering=False, debug=False,
                   num_devices=cfg.C, num_swdge_queues=cfg.NQ)

    # I/O ------------------------------------------------------------------
    xtab = nc.dram_tensor("xtab", [NP, D], f32, kind="ExternalInput")
    xrows = nc.dram_tensor("xrows", [P, W * D], f32, kind="ExternalInput")
    tmp_in = nc.dram_tensor("temp", [1, 4], f32, kind="ExternalInput")
    blobs = {}
    for name, g in graphs.items():
        blobs[name] = dict(
            gi=nc.dram_tensor(f"gi_{name}", [P, g.total // 16],
                              mybir.dt.int16, kind="ExternalInput"),
            ld=nc.dram_tensor(f"ld_{name}", [P, g.total // P], f32,
                              kind="ExternalInput"),
            wv=nc.dram_tensor(f"wv_{name}", [P, g.total // P], f32,
                              kind="ExternalInput"),
        )
    iota_in = nc.dram_tensor("iota", [P, P], f32, kind="ExternalInput")
    out_dev = nc.dram_tensor("out_dev", [P, W * D], f32,
                             kind="ExternalOutput")
    zpos_dev = nc.dram_tensor("zpos_dev", [P, W * D], f32,
                              kind="ExternalOutput")
    zneg_dev = nc.dram_tensor("zneg_dev", [P, W * D], f32,
                              kind="ExternalOutput")

    rg = [list(range(cfg.C))]
    mx = max(g.max_cell for g in graphs.values())
    mxb = max(g.max_blk for g in graphs.values())

    with tile.TileContext(nc) as tc:
        with (
            tc.tile_pool(name="const", bufs=1) as constp,
            tc.tile_pool(name="acc", bufs=1) as accp,
            tc.tile_pool(name="blob", bufs=2) as blobp,
            tc.tile_pool(name="vg", bufs=2) as vp,
            tc.tile_pool(name="sm", bufs=2) as sp,
            tc.tile_pool(name="ps", bufs=8, space="PSUM") as pp,
            tc.tile_pool(name="dram", bufs=1, space="DRAM") as dp,
        ):
            iota_t = constp.tile([P, P], f32, name="iota_t")
            nc.sync.dma_start(iota_t[:], iota_in[:])
            xr = constp.tile([P, W * D], f32, name="xr")
            nc.sync.dma_start(xr[:], xrows[:])
            lxr = constp.tile([P, W * D], f32, name="lxr")
            outr = constp.tile([P, W * D], f32, name="outr")
            acc = accp.tile([P, W * D], f32, name="acc_t")

            # temp coefficients -> [128, 3] broadcast tile
            tco = constp.tile([1, 4], f32, name="tco")
            nc.sync.dma_start(tco[:], tmp_in[:])
            nc.vector.tensor_scalar_max(tco[:], tco[:], 0.0)  # relu
            co = constp.tile([1, 4], f32, name="co")
            # co0 = T0 ; co1 = T1-T0 ; co2 = (T0+T2-2*T1)/4
            nc.vector.tensor_copy(co[:, 0:1], tco[:, 0:1])
            nc.vector.tensor_tensor(co[:, 1:2], tco[:, 1:2], tco[:, 0:1],
                                    op=mybir.AluOpType.subtract)
            nc.vector.tensor_tensor(co[:, 2:3], tco[:, 0:1], tco[:, 2:3],
                                    op=mybir.AluOpType.add)
            nc.vector.tensor_scalar(co[:, 3:4], tco[:, 1:2], -2.0, None,
                                    op0=mybir.AluOpType.mult)
            nc.vector.tensor_tensor(co[:, 2:3], co[:, 2:3], co[:, 3:4],
                                    op=mybir.AluOpType.add)
            nc.vector.tensor_scalar(co[:, 2:3], co[:, 2:3], 0.25, None,
                                    op0=mybir.AluOpType.mult)
            # broadcast [1,4] coeffs to all partitions: ones[1,128].T @ co
            ones1 = constp.tile([1, P], f32, name="ones1")
            nc.vector.memset(ones1[:], 1.0)
            cps = pp.tile([P, 4], f32, tag="psw", name="cps")
            nc.tensor.matmul(cps[:], ones1[:], co[:], start=True, stop=True)
            cob = constp.tile([P, 4], f32, name="cob")
            nc.vector.tensor_copy(cob[:], cps[:])

            qcnt = [0]

            def spmm(g, blob, table_ap):
                """One SpMM stage: result lands in `acc`."""
                for ws, cells in g.blocks:
                    blk_n = sum(n for (_, _, n, _) in cells)
                    if blk_n == 0:
                        continue
                    b_off = cells[0][1]
                    gt = blobp.tile([P, mxb // 16], mybir.dt.int16, tag="gt")
                    lt = blobp.tile([P, mxb // P], f32, tag="lt")
                    wt_ = blobp.tile([P, mxb // P], f32, tag="wt")
                    nc.sync.dma_start(
                        gt[:, : blk_n // 16],
                        blob["gi"][:, b_off // 16:(b_off + blk_n) // 16])
                    nc.sync.dma_start(
                        lt[:, : blk_n // P],
                        blob["ld"][:, b_off // P:(b_off + blk_n) // P])
                    nc.sync.dma_start(
                        wt_[:, : blk_n // P],
                        blob["wv"][:, b_off // P:(b_off + blk_n) // P])
                    ptiles = {w: pp.tile([P, D], f32, tag="psw",
                                         name=f"ps_{w}") for w in ws}
                    touched = set()
                    for (c, off, n, wtl) in cells:
                        if n == 0:
                            continue
                        nt = n // P
                        lo = off - b_off
                        V = vp.tile([P, mx // P, D], f32, tag="V")
                        GCAP = 1024  # SWDGE ring: ≤1024 idx per gather
                        for g0 in range(0, n, GCAP):
                            gn = min(GCAP, n - g0)
                            nc.gpsimd.dma_gather(
                                V[:, g0 // P:(g0 + gn) // P, :],
                                table_ap[c * CHUNK:(c + 1) * CHUNK, :],
                                gt[:, (lo + g0) // 16:(lo + g0 + gn) // 16],
                                gn, gn, D, queue_num=qcnt[0] % cfg.NQ,
                                single_packet=False)
                            qcnt[0] += 1
                        S = sp.tile([P, (mx // P) * P], f32, tag="S")
                        s3 = S[:].rearrange("p (t r) -> p t r", r=P)[:, :nt, :]
                        ldb = lt[:, lo // P:(lo + n) // P] \
                            .to_broadcast([P, nt, P])
                        iob = iota_t[:].unsqueeze(1).to_broadcast([P, nt, P])
                        nc.vector.tensor_tensor(s3, iob, ldb,
                                                op=mybir.AluOpType.is_equal)
                        if V_MUL:
                            wvb = wt_[:, lo // P:(lo + n) // P] \
                                .to_broadcast([P, nt, D])
                            nc.vector.tensor_tensor(
                                V[:, :nt, :], V[:, :nt, :], wvb,
                                op=mybir.AluOpType.mult)
                        else:
                            wvb = wt_[:, lo // P:(lo + n) // P] \
                                .to_broadcast([P, nt, P])
                            nc.vector.tensor_tensor(
                                s3, s3, wvb, op=mybir.AluOpType.mult)
                        ti = 0
                        for (w, t) in wtl:
                            for j in range(t):
                                last = (c == max(
                                    cx for (cx, _, nx, wl) in cells
                                    if nx and any(wx == w for wx, _ in wl))
                                    and j == t - 1)
                                nc.tensor.matmul(
                                    ptiles[w][:],
                                    S[:, (ti + j) * P:(ti + j + 1) * P],
                                    V[:, ti + j, :],
                                    start=(w not in touched),
                                    stop=last)
                                touched.add(w)
                            ti += t
                    for w in ws:
                        nc.any.tensor_copy(acc[:, w * D:(w + 1) * D],
                                           ptiles[w][:])

            do_s1 = variant != "empty"
            do_rest = variant in ("noag", "full")
            do_ag = variant == "full"
            for _rep in range(repeat):
                ts = mybir.AluOpType
                if not do_s1:
                    nc.vector.memset(acc[:], 0.0)
                if not do_rest:
                    nc.vector.memset(outr[:], 0.0)
                # stage 1: sp1 = Anorm @ x ; Lx = x - sp1
                if do_s1:
                    spmm(graphs["L"], blobs["L"], xtab[:])
                    nc.vector.tensor_tensor(lxr[:], xr[:], acc[:],
                                            op=ts.subtract)
                bounce1 = dp.tile([cfg.RP, D], f32, name="bn_lx")
                t_lx = dp.tile([NP, D], f32, addr_space="Shared",
                               name="tb_lx")
                if do_s1:
                    nc.sync.dma_start(
                        bounce1[:].rearrange("(p w) d -> p (w d)", p=P),
                        lxr[:])
                if do_ag:
                    nc.gpsimd.collective_compute(
                        "AllGather", ts.bypass, replica_groups=rg,
                        ins=[bounce1[:].opt()], outs=[t_lx[:].opt()])

                # stage 2: sp2 = Anorm @ Lx ; LLx = Lx - sp2 ; out = combo
                bounce2 = dp.tile([cfg.RP, D], f32, name="bn_out")
                t_out = dp.tile([NP, D], f32, addr_space="Shared",
                                name="tb_out")
                if do_rest:
                    spmm(graphs["L"], blobs["L"], t_lx[:])
                    nc.vector.tensor_tensor(acc[:], lxr[:], acc[:],
                                            op=ts.subtract)
                    nc.vector.tensor_scalar(acc[:], acc[:], cob[:, 2:3],
                                            None, op0=ts.mult)
                    nc.vector.tensor_scalar(outr[:], xr[:], cob[:, 0:1],
                                            None, op0=ts.mult)
                    nc.vector.tensor_tensor(outr[:], outr[:], acc[:],
                                            op=ts.add)
                    nc.vector.tensor_scalar(lxr[:], lxr[:], cob[:, 1:2],
                                            None, op0=ts.mult)
                    nc.vector.tensor_tensor(outr[:], outr[:], lxr[:],
                                            op=ts.add)
                nc.sync.dma_start(out_dev[:], outr[:])
                if do_rest:
                    nc.sync.dma_start(
                        bounce2[:].rearrange("(p w) d -> p (w d)", p=P),
                        outr[:])
                if do_ag:
                    nc.gpsimd.collective_compute(
                        "AllGather", ts.bypass, replica_groups=rg,
                        ins=[bounce2[:].opt()], outs=[t_out[:].opt()])

                # stage 3: z1 = NB @ out
                bounce3 = dp.tile([cfg.RP, D], f32, name="bn_z1")
                t_z1 = dp.tile([NP, D], f32, addr_space="Shared",
                               name="tb_z1")
                if do_rest:
                    spmm(graphs["NB"], blobs["NB"], t_out[:])
                    nc.sync.dma_start(
                        bounce3[:].rearrange("(p w) d -> p (w d)", p=P),
                        acc[:])
                if do_ag:
                    nc.gpsimd.collective_compute(
                        "AllGather", ts.bypass, replica_groups=rg,
                        ins=[bounce3[:].opt()], outs=[t_z1[:].opt()])

                # stage 5: v1 = NS @ out  (shuf-composed)
                bounce5 = dp.tile([cfg.RP, D], f32, name="bn_v1")
                t_v1 = dp.tile([NP, D], f32, addr_space="Shared",
                               name="tb_v1")
                if do_rest:
                    spmm(graphs["NS"], blobs["NS"], t_out[:])
                    nc.sync.dma_start(
                        bounce5[:].rearrange("(p w) d -> p (w d)", p=P),
                        acc[:])
                if do_ag:
                    nc.gpsimd.collective_compute(
                        "AllGather", ts.bypass, replica_groups=rg,
                        ins=[bounce5[:].opt()], outs=[t_v1[:].opt()])

                # stage 4: z_pos = NB @ z1
                if do_rest:
                    spmm(graphs["NB"], blobs["NB"], t_z1[:])
                nc.sync.dma_start(zpos_dev[:], acc[:])

                # stage 6: z_neg = NB @ v1
                if do_rest:
                    spmm(graphs["NB"], blobs["NB"], t_v1[:])
                nc.sync.dma_start(zneg_dev[:], acc[:])

    nc.compile()
    return nc


# ----------------------------------------------------------------- driver --

def _prep(cfg, x, shuf, edge_index, edge_weight, nb_index, nb_weight):
    row = edge_index[0].astype(np.int64)
    col = edge_index[1].astype(np.int64)
    ew = edge_weight.astype(np.float32)
    deg = np.zeros(cfg.N, np.float32)
    np.add.at(deg, row, ew)
    dis = np.where(deg > 0, 1.0 / np.sqrt(np.maximum(deg, 1e-30)), 0.0) \
        .astype(np.float32)
    w_norm = dis[row] * ew * dis[col]
    nrow = nb_index[0].astype(np.int64)
    ncol = nb_index[1].astype(np.int64)
    nwv = nb_weight.astype(np.float32)
    scol = shuf.astype(np.int64)[ncol]
    gL = Graph(cfg, row, col, w_norm)
    gNB = Graph(cfg, nrow, ncol, nwv)
    gNS = Graph(cfg, nrow, scol, nwv)
    return gL, gNB, gNS


def run_pipeline(cfg, x, shuf, edge_index, edge_weight, nb_index, nb_weight,
                 temp, trace=False):
    from concourse.bass_utils import run_bass_kernel_spmd

    x = np.asarray(x, np.float32)
    gL, gNB, gNS = _prep(cfg, x, np.asarray(shuf), np.asarray(edge_index),
                         np.asarray(edge_weight), np.asarray(nb_index),
                         np.asarray(nb_weight))
    graphs = {"L": gL, "NB": gNB, "NS": gNS}
    nc = build_program(cfg, graphs)

    xdev = _to_dev_table(cfg, x)
    iota = np.tile(np.arange(P, dtype=np.float32), (P, 1))
    tmp4 = np.zeros((1, 4), np.float32)
    tmp4[0, :3] = np.asarray(temp, np.float32)
    in_maps = []
    for k in range(cfg.C):
        xr_k = xdev[k * cfg.RP:(k + 1) * cfg.RP].reshape(P, cfg.W * cfg.D)
        m = {"xtab": xdev, "xrows": xr_k, "temp": tmp4, "iota": iota}
        for name, g in graphs.items():
            m[f"gi_{name}"] = g.gidx[k]
            m[f"ld_{name}"] = g.ldw[k]
            m[f"wv_{name}"] = g.wvv[k]
        in_maps.append(m)
    res = run_bass_kernel_spmd(nc, in_maps, core_ids=list(range(cfg.C)),
                               trace=trace)
    outs, zps, zns = [], [], []
    for k in range(cfg.C):
        outs.append(_from_dev_rows(cfg, res.results[k]["out_dev"]))
        zps.append(_from_dev_rows(cfg, res.results[k]["zpos_dev"]))
        zns.append(_from_dev_rows(cfg, res.results[k]["zneg_dev"]))
    out = (np.concatenate(outs), np.concatenate(zps), np.concatenate(zns))
    return (out, res) if trace else (out, res)


def make_runner(nc, in_maps, n_cores):
    """Device-resident repeated-execution runner for timing (axon path)."""
    import jax
    import numpy as jnp_np
    from jax.experimental.shard_map import shard_map
    from jax.sharding import Mesh, NamedSharding, PartitionSpec

    import concourse.mybir as mybir
    from concourse import bass2jax as bj

    bj.install_neuronx_cc_hook()
    partition_name = (nc.partition_id_tensor.name
                      if nc.partition_id_tensor else None)
    in_names, out_names, out_avals, zero_outs = [], [], [], []
    for alloc in nc.m.functions[0].allocations:
        if not isinstance(alloc, mybir.MemoryLocationSet):
            continue
        name = alloc.memorylocations[0].name
        if alloc.kind == "ExternalInput":
            if name != partition_name:
                in_names.append(name)
        elif alloc.kind == "ExternalOutput":
            shape = tuple(alloc.tensor_shape)
            dtype = mybir.dt.np(alloc.dtype)
            out_names.append(name)
            out_avals.append(jax.core.ShapedArray(shape, dtype))
            zero_outs.append(np.zeros(shape, dtype))
    n_params = len(in_names)
    in_names.extend(out_names)
    if partition_name is not None:
        in_names.append(partition_name)

    def _body(*args):
        operands = list(args)
        if partition_name is not None:
            operands.append(bj.partition_id_tensor())
        outs = bj._bass_exec_p.bind(
            *operands, out_avals=tuple(out_avals),
            in_names=tuple(in_names), out_names=tuple(out_names),
            lowering_input_output_aliases=(),
            sim_require_finite=True, sim_require_nnan=True, nc=nc)
        return tuple(outs)

    devices = jax.devices()[:n_cores]
    mesh = Mesh(np.asarray(devices), ("core",))
    spec = PartitionSpec("core")
    nio = n_params + len(out_names)
    fn = jax.jit(shard_map(_body, mesh=mesh, in_specs=(spec,) * nio,
                           out_specs=(spec,) * len(out_names),
                           check_rep=False), keep_unused=True)
    concat = [np.concatenate([np.asarray(m[nm]) for m in in_maps])
              for nm in in_names[:n_params]]
    concat += [np.zeros((n_cores * z.shape[0], *z.shape[1:]), z.dtype)
               for z in zero_outs]
    sh = NamedSharding(mesh, spec)
    dev_in = [jax.device_put(a, sh) for a in concat]
    return fn, dev_in, out_names, out_avals


def timed_pipeline(cfg, x, shuf, edge_index, edge_weight, nb_index,
                   nb_weight, temp, iters=10, repeat=1, variant="full"):
    import time as _time

    import jax

    x = np.asarray(x, np.float32)
    gL, gNB, gNS = _prep(cfg, x, np.asarray(shuf), np.asarray(edge_index),
                         np.asarray(edge_weight), np.asarray(nb_index),
                         np.asarray(nb_weight))
    graphs = {"L": gL, "NB": gNB, "NS": gNS}
    print("[timed] building program...", flush=True)
    nc = build_program(cfg, graphs, repeat=repeat, variant=variant)
    print("[timed] program built", flush=True)
    xdev = _to_dev_table(cfg, x)
    iota = np.tile(np.arange(P, dtype=np.float32), (P, 1))
    tmp4 = np.zeros((1, 4), np.float32)
    tmp4[0, :3] = np.asarray(temp, np.float32)
    in_maps = []
    for k in range(cfg.C):
        m = {"xtab": xdev,
             "xrows": xdev[k * cfg.RP:(k + 1) * cfg.RP]
             .reshape(P, cfg.W * cfg.D),
             "temp": tmp4, "iota": iota}
        for name, g in graphs.items():
            m[f"gi_{name}"] = g.gidx[k]
            m[f"ld_{name}"] = g.ldw[k]
            m[f"wv_{name}"] = g.wvv[k]
        in_maps.append(m)
    fn, dev_in, out_names, out_avals = make_runner(nc, in_maps, cfg.C)
    print("[timed] inputs on device, warming up...", flush=True)
    r = fn(*dev_in)
    jax.block_until_ready(r)       # warmup / compile
    print("[timed] warmup done", flush=True)
    t0 = _time.time()
    for _ in range(iters):
        r = fn(*dev_in)
    jax.block_until_ready(r)
    dt_pipe = (_time.time() - t0) / iters
    t0 = _time.time()
    for _ in range(3):
        r = fn(*dev_in)
        jax.block_until_ready(r)
    dt_sync = (_time.time() - t0) / 3
    outs = []
    for i, name in enumerate(out_names):
        arr = np.asarray(r[i]).reshape(cfg.C, *out_avals[i].shape)
        outs.append({name: arr})
    res = {name: np.concatenate(
        [_from_dev_rows(cfg, np.asarray(r[i]).reshape(
            cfg.C, *out_avals[i].shape)[k]) for k in range(cfg.C)])
        for i, name in enumerate(out_names)}
    out = (res["out_dev"], res["zpos_dev"], res["zneg_dev"])
    return out, dt_pipe, dt_sync


def kernel(x, shuf, edge_index, edge_weight, nb_index, nb_weight, temp):
    out, _ = run_pipeline(Cfg(), x, shuf, edge_index, edge_weight,
                          nb_index, nb_weight, temp)
    return out

